# revision 30
# baseline (speedup 1.0000x reference)
"""Multi-head self-attention on 8 Trainium2 NeuronCores.

Sharding: tensor-parallel over heads (2 heads per core, both batch elements
on every core). Each core computes qkv projection / attention / its slice of
the output projection (rows of W_out for its heads), producing a partial
[B, N, D] output (bf16); the host sums the 8 partials and adds b_out.

Per-core dataflow (layouts chosen so no engine ever needs a cross-partition
shift except via DMA):
  - host supplies x^T [B, D, N] so the QKV projection can run directly
    (contraction dim on partitions for both operands)
  - QKV^T = Wsel^T @ x^T -> Q^T, K^T, V^T, each [128=2*64 head rows, N]
  - V^T is PE-transposed back to V [k, e] chunks, with a LEADING ones column
    per head so the P@V matmul also produces the softmax row-sums, landing
    on psum partition 0 where the gpsimd broadcast can read them directly
  - S^T = K^T(head)^T-block @ Q^T (contraction = head dim 64). The two
    heads' S matmuls write one shared [128, 2, 512] psum tile and carry
    disjoint PE row-tiles (auto tile_position (0,0)/(64,0)), so the
    hardware runs them concurrently and one exp covers both heads.
  - P^T = exp(S^T / sqrt(dp)) fused in the PSUM->SBUF evacuation on ScalarE
    (no max subtraction: scores are ~N(0,1), exp is safe in fp32)
  - O^T_aug = [1|V]^T-block @ P^T -> row 0 = softmax denominator s, rows
    1..64 = unnormalized O^T; normalize via fast-approx reciprocal + gpsimd
    partition_broadcast + multiply, DMA-shift into the combined O^T tile
  - y_partial = O^T-block^T @ W_out_slice

QCH=512 keeps every psum user at 1-2 banks: 4 banks S double-buffer +
2 banks PV accumulators + 2 banks for filler work (projection blocks and
the deferred QKV pieces), so fillers never contend with the S ring.
Deferred prep is deadline-scheduled into the attention windows.
"""

import numpy as np
import ml_dtypes

B, N, D, H, DP = 2, 2048, 1024, 16, 64
SCALE = float(DP) ** 0.5
NCORES = 8
HC = H // NCORES            # heads per core = 2
E = HC * DP                 # per-core head-dim total = 128
QCH = 512                   # q columns handled per attention chunk
NQ = N // QCH               # 4
KB = N // 128               # 16 k blocks
DC = D // 128               # 8 contraction chunks for the qkv projection

BF16 = ml_dtypes.bfloat16

_CACHE = {}


def _build_bass(with_bias=False):
    import concourse.bass as bass
    import concourse.mybir as mybir
    import concourse.tile as tile
    from concourse import bacc
    from concourse.masks import make_identity

    MM_DT = mybir.dt.bfloat16    # matmul input dtype
    P_DT = mybir.dt.bfloat16     # exp(S^T) storage dtype
    F32 = mybir.dt.float32

    # nonzero b_qkv is handled by an extra contraction chunk whose x^T rows
    # are [ones, 0...] and whose weight rows carry the bias (bias as matmul)
    DCX = DC + (1 if with_bias else 0)
    VAW = 130  # VA free width: 2 heads x [ones | V(64)]
    RING = 6   # P^T ring depth (PV runs at lag 3-4 in window pairs)
    nc = bacc.Bacc(None, target_bir_lowering=False)
    xt = nc.dram_tensor("xt", [B, DCX * 128, N], MM_DT, kind="ExternalInput")[:]
    wsel = nc.dram_tensor("wsel", [DCX * 128, 3 * E], MM_DT, kind="ExternalInput")[:]
    wout = nc.dram_tensor("wout", [E, D], MM_DT, kind="ExternalInput")[:]
    # bf16 partials halve the output DMA; the host sums in fp32
    y = nc.dram_tensor("y", [B, N, D], MM_DT, kind="ExternalOutput")[:]

    with tile.TileContext(nc) as tc:
        with (
            tc.tile_pool(name="consts", bufs=1) as consts,
            tc.tile_pool(name="xtp", bufs=2) as xtp,
            tc.tile_pool(name="ptp", bufs=4) as ptp,
            tc.tile_pool(name="qkvp", bufs=2) as qkvp,
            tc.tile_pool(name="vap", bufs=2) as vap,
            tc.tile_pool(name="otp", bufs=2) as otp,
            tc.tile_pool(name="evacp", bufs=2) as evacp,
            tc.tile_pool(name="normp", bufs=2) as normp,
            # 8 psum banks total: paired-S 2x[128,2,512]f32 (4) +
            # pv accumulators 2x[65,512] (2) + filler scratch 2x[128,512] (2)
            tc.tile_pool(name="ps_s", bufs=2, space="PSUM") as ps_s,
            tc.tile_pool(name="ps_g", bufs=2, space="PSUM") as ps_g,
            tc.tile_pool(name="ps_y", bufs=2, space="PSUM") as ps_y,
        ):
            # DMA issue order matters: the Sync engine issues serially and the
            # first QKV matmul waits on xt[b=0] chunk 0 + WS, so those go
            # first; nk-halved transfers let the nk=0 projections (all the
            # attention start needs) complete in half the bytes
            XTs = []
            for b in range(B):
                XTs.append(xtp.tile([128, DCX, N], MM_DT, tag="xt", name="xt"))
            WS = consts.tile([128, DCX, 3 * E], MM_DT)
            wsr = wsel.rearrange("(dc p) e -> p dc e", p=128)
            xtb0 = xt[0].rearrange("(dc p) n -> p dc n", p=128)
            for dc in range(DCX):
                nc.sync.dma_start(out=XTs[0][:, dc, 0:1024], in_=xtb0[:, dc, 0:1024])
                nc.sync.dma_start(out=WS[:, dc, :], in_=wsr[:, dc, :])
            xtb1 = xt[1].rearrange("(dc p) n -> p dc n", p=128)
            for dc in range(DCX):
                nc.sync.dma_start(out=XTs[0][:, dc, 1024:], in_=xtb0[:, dc, 1024:])
            for nk in range(2):
                for dc in range(DCX):
                    nc.sync.dma_start(
                        out=XTs[1][:, dc, nk * 1024 : (nk + 1) * 1024],
                        in_=xtb1[:, dc, nk * 1024 : (nk + 1) * 1024],
                    )
            WOUT = consts.tile([128, D], MM_DT)
            nc.sync.dma_start(out=WOUT, in_=wout)
            IDENT = consts.tile([128, 128], MM_DT)
            make_identity(nc, IDENT)
            WARM = consts.tile([1, 1], F32)
            nc.vector.memset(WARM, 0.0)
            nc.scalar.activation(
                out=WARM, in_=WARM, func=mybir.ActivationFunctionType.Exp
            )
            # p-state warmup: the PE clock ramps 0.65 -> 1.2 -> 2.4 GHz after
            # ~3us of continuous busy. Junk transposes while the first x^T /
            # W chunks stream in mean the real matmuls start at full clock.
            WARMPS = ps_g.tile([128, 128], MM_DT, tag="g", name="warm_ps")
            for _ in range(24):
                nc.tensor.transpose(WARMPS, IDENT, IDENT)

            QKVTs, VAs = [], []
            fillers = []  # deferred projection sub-blocks (no deadline)
            for b in range(B):
                QKVTs.append(
                    [
                        qkvp.tile([128, N], MM_DT, tag=f"qkv{eb}", name=f"qkv{eb}")
                        for eb in range(3)
                    ]
                )
                # V chunks with a LEADING ones column: [1 | V_h0(64) | 1 | V_h1]
                VA = vap.tile([128, KB, VAW], MM_DT, tag="va", name="va")
                nc.gpsimd.memset(VA[:, :, 0:1], 1.0)
                nc.gpsimd.memset(VA[:, :, VAW // 2 : VAW // 2 + 1], 1.0)
                VAs.append(VA)

            def emit_qkv_half(b2, eb, nk, hf, pool):
                # half of a projection block: one 512-col chunk of QKV^T
                tag = "g" if pool is ps_g else "q"
                ps = pool.tile([128, 512], F32, tag=tag, name="ps_qkv")
                c0 = nk * 1024 + hf * 512
                for dc in range(DCX):
                    nc.tensor.matmul(
                        ps,
                        lhsT=WS[:, dc, eb * 128 : (eb + 1) * 128],
                        rhs=XTs[b2][:, dc, c0 : c0 + 512],
                        start=(dc == 0),
                        stop=(dc == DCX - 1),
                    )
                nc.vector.tensor_copy(
                    out=QKVTs[b2][eb][:, c0 : c0 + 512], in_=ps
                )

            def emit_vtrans(b2, kc, pool):
                tag = "g" if pool is ps_g else "q"
                pst = pool.tile([128, 128], MM_DT, tag=tag, name="ps_vt")
                VT2 = QKVTs[b2][2]
                VA2 = VAs[b2]
                nc.tensor.transpose(
                    pst, VT2[:, kc * 128 : (kc + 1) * 128], IDENT
                )
                nc.vector.tensor_copy(out=VA2[:, kc, 1 : 1 + DP], in_=pst[:, 0:DP])
                nc.vector.tensor_copy(
                    out=VA2[:, kc, VAW // 2 + 1 : VAW // 2 + 1 + DP],
                    in_=pst[:, DP : 2 * DP],
                )

            # ---- deferred-prep schedule. Window index W counts kc windows
            # globally (16 per qh, 64 per batch). Each prep item carries the
            # last window index at which it may be emitted (one before its
            # first reader); pops happen at the top of each window, so a
            # deadline of W is safe for readers inside window W+1.
            def deadlines(base, items):
                return [(base + dl, it) for dl, it in items]

            prep = []
            for b2 in range(B):
                base = 64 * b2
                items = []
                for nk in range(2):
                    for hf in range(2):
                        qi = 2 * nk + hf  # qh index using these Q cols
                        items.append(
                            (16 * qi - 2,
                             lambda b2=b2, nk=nk, hf=hf: emit_qkv_half(
                                 b2, 0, nk, hf, ps_y))
                        )
                        kf = 8 * nk + 4 * hf  # first k-block in these cols
                        items.append(
                            (kf - 2,
                             lambda b2=b2, nk=nk, hf=hf: emit_qkv_half(
                                 b2, 1, nk, hf, ps_y))
                        )
                        items.append(
                            (kf - 1,
                             lambda b2=b2, nk=nk, hf=hf: emit_qkv_half(
                                 b2, 2, nk, hf, ps_y))
                        )
                for kc in range(KB):
                    # first PV reader of VA[kc] is emitted in window kc+3
                    items.append(
                        (kc - 1,
                         lambda b2=b2, kc=kc: emit_vtrans(b2, kc, ps_y))
                    )
                prep.extend(deadlines(base, items))
            prep.sort(key=lambda it: it[0])

            # b=0 items that would be due before the attention loop begins
            # run serially now (ps_g is free until the first pv allocation)
            while prep and prep[0][0] < 1:
                _, it = prep.pop(0)
                it()

            def emit_proj_block(spec, pool=None):
                b2, OT2, nb = spec
                pool = ps_y if pool is None else pool
                tag = "g" if pool is ps_g else "q"
                ysb = evacp.tile([128, D], MM_DT, tag="y", name="ysb", bufs=4)
                for dc2 in range(D // 512):
                    py = pool.tile([128, 512], F32, tag=tag, name="py")
                    nc.tensor.matmul(
                        py,
                        lhsT=OT2[:, nb * 128 : (nb + 1) * 128],
                        rhs=WOUT[:, dc2 * 512 : (dc2 + 1) * 512],
                        start=True,
                        stop=True,
                    )
                    nc.vector.tensor_copy(
                        out=ysb[:, dc2 * 512 : (dc2 + 1) * 512], in_=py
                    )
                nc.sync.dma_start(
                    out=y[b2, nb * 128 : (nb + 1) * 128, :], in_=ysb
                )

            # ---- phase 2: attention
            W = 0  # global window counter
            for b in range(B):
                QT, KT, VT = QKVTs[b]
                VA = VAs[b]
                OT = otp.tile([128, N], MM_DT, tag="ot", name="ot")
                for qh in range(NQ):
                    PT = ptp.tile(
                        [128, RING, HC, 512], P_DT, tag="pt", name="pt"
                    )
                    # one 1-bank PV accumulator per head, held across kc
                    pvs = [
                        ps_g.tile([DP + 1, 512], F32, tag="g", name=f"pv{h}")
                        for h in range(HC)
                    ]

                    def pv_mms(kc):
                        for h in range(HC):
                            nc.tensor.matmul(
                                pvs[h],
                                lhsT=VA[
                                    :, kc,
                                    h * (VAW // 2) : h * (VAW // 2) + DP + 1,
                                ],
                                rhs=PT[:, kc % RING, h, :],
                                start=(kc == 0),
                                stop=(kc == KB - 1),
                            )

                    # windows processed in PAIRS: two PV pairs back-to-back,
                    # then two S pairs back-to-back. Each S<->PV transition
                    # costs ~270ns of weight-register serialization, so
                    # halving the transition count saves ~180ns/window.
                    for kc2 in range(0, KB, 2):
                        for kcp in (kc2 - 4, kc2 - 3):
                            if kcp >= 0:
                                pv_mms(kcp)
                        # mandatory deadline pops, then one opportunistic pop
                        popped = False
                        while prep and prep[0][0] <= W:
                            prep.pop(0)[1]()
                            popped = True
                        if not popped and prep and kc2 >= 2:
                            prep.pop(0)[1]()
                        elif not popped and fillers and kc2 >= 2:
                            fillers.pop(0)()
                        # both heads' S matmuls share one psum tile and
                        # disjoint PE row-tiles -> hardware runs them
                        # concurrently; one exp evacuates both
                        q0 = qh * QCH
                        for kc in (kc2, kc2 + 1):
                            ps2 = ps_s.tile(
                                [128, HC, 512], F32, tag="s", name="s2"
                            )
                            for h in range(HC):
                                nc.tensor.matmul(
                                    ps2[:, h, :],
                                    lhsT=KT[
                                        h * DP : (h + 1) * DP,
                                        kc * 128 : (kc + 1) * 128,
                                    ],
                                    rhs=QT[h * DP : (h + 1) * DP, q0 : q0 + 512],
                                    start=True,
                                    stop=True,
                                )
                            nc.scalar.activation(
                                out=PT[:, kc % RING, :, :],
                                in_=ps2,
                                func=mybir.ActivationFunctionType.Exp,
                                scale=1.0 / SCALE,
                            )
                            W += 1
                    while prep and prep[0][0] <= W:
                        prep.pop(0)[1]()
                    for kcp in range(KB - 4, KB):
                        pv_mms(kcp)

                    # normalize: denominator row is psum partition 0 (leading
                    # ones column). Evacuate pv on the scalar engine (slack at
                    # every boundary; frees the psum bank immediately), then
                    # fast-approx reciprocal, gpsimd broadcast, multiply
                    # (rows 0..64 for base-partition alignment; row 0 unused),
                    # DMA-shift into O^T.
                    for h in range(HC):
                        pv = pvs[h]
                        ocp = normp.tile([DP + 1, QCH], F32, tag="ocp", name="ocp")
                        nc.vector.tensor_copy(out=ocp, in_=pv)
                        rt = normp.tile([1, QCH], F32, tag="rt", name="rt")
                        nc.vector.reciprocal_approx_fast(out=rt, in_=ocp[0:1, :])
                        bc = normp.tile([DP + 1, QCH], F32, tag="bc", name="bc")
                        nc.gpsimd.partition_broadcast(bc, rt)
                        ots = normp.tile([DP + 1, QCH], MM_DT, tag="ots", name="ots")
                        nc.vector.tensor_mul(out=ots, in0=ocp, in1=bc)
                        nc.sync.dma_start(
                            out=OT[h * DP : (h + 1) * DP, qh * QCH : (qh + 1) * QCH],
                            in_=ots[1 : DP + 1, :],
                        )

                    # queue this qh's projection blocks as fillers (their
                    # norm-chain inputs are ready well before they are popped)
                    for nb in range(qh * QCH // 128, (qh + 1) * QCH // 128):
                        fillers.append(
                            (lambda pool=None, s=(b, OT, nb): emit_proj_block(s, pool))
                        )

            # drain remaining fillers; pv accumulators are dead, so alternate
            # psum pools to keep 4 blocks in flight
            di = 0
            while fillers:
                fillers.pop(0)(ps_g if di % 2 else ps_y)
                di += 1
    nc.finalize()
    return nc


def _get_bass(with_bias=False):
    key = f"nc{int(with_bias)}"
    if key not in _CACHE:
        _CACHE[key] = _build_bass(with_bias)
    return _CACHE[key]


def _make_in_maps(x, W_qkv, b_qkv, W_out):
    """Shard the full inputs into the 8 per-core input dicts."""
    x = np.asarray(x, dtype=np.float32)
    W_qkv = np.asarray(W_qkv, dtype=np.float32)
    b_qkv = np.asarray(b_qkv, dtype=np.float32)
    W_out = np.asarray(W_out, dtype=np.float32)

    with_bias = bool(np.any(b_qkv))
    # x^T per batch, shared by all cores (+ optional bias chunk rows)
    xtt = x.transpose(0, 2, 1)
    if with_bias:
        aug = np.zeros((B, 128, N), dtype=np.float32)
        aug[:, 0, :] = 1.0
        xtt = np.concatenate([xtt, aug], axis=1)
    xt = np.ascontiguousarray(xtt).astype(BF16)

    in_maps = []
    for c in range(NCORES):
        heads = [HC * c + i for i in range(HC)]
        # W_qkv columns: head h occupies cols [h*3*DP, (h+1)*3*DP) as [q|k|v]
        qcols = [W_qkv[:, h * 3 * DP : h * 3 * DP + DP] for h in heads]
        kcols = [W_qkv[:, h * 3 * DP + DP : h * 3 * DP + 2 * DP] for h in heads]
        vcols = [W_qkv[:, h * 3 * DP + 2 * DP : h * 3 * DP + 3 * DP] for h in heads]
        wsel = np.concatenate(qcols + kcols + vcols, axis=1)  # [D, 3*E]
        if with_bias:
            bq = [b_qkv[h * 3 * DP : h * 3 * DP + DP] for h in heads]
            bk = [b_qkv[h * 3 * DP + DP : h * 3 * DP + 2 * DP] for h in heads]
            bv = [b_qkv[h * 3 * DP + 2 * DP : h * 3 * DP + 3 * DP] for h in heads]
            brow = np.concatenate(bq + bk + bv)  # [3*E]
            baug = np.zeros((128, 3 * E), dtype=np.float32)
            baug[0, :] = brow
            wsel = np.concatenate([wsel, baug], axis=0)
        woutc = np.concatenate(
            [W_out[h * DP : (h + 1) * DP, :] for h in heads], axis=0
        )  # [E, D]
        in_maps.append(
            {
                "xt": xt,
                "wsel": np.ascontiguousarray(wsel).astype(BF16),
                "wout": np.ascontiguousarray(woutc).astype(BF16),
            }
        )
    return in_maps, with_bias


def _run(in_maps, with_bias=False, trace=False):
    from concourse import bass_utils

    nc = _get_bass(with_bias)
    return bass_utils.run_bass_kernel_spmd(
        nc, in_maps, core_ids=list(range(NCORES)), trace=trace
    )


def kernel(x, W_qkv, b_qkv, W_out, b_out, _trace=False):
    in_maps, with_bias = _make_in_maps(x, W_qkv, b_qkv, W_out)
    res = _run(in_maps, with_bias=with_bias, trace=_trace)
    y = np.zeros((B, N, D), dtype=np.float32)
    for r in res.results:
        y += np.asarray(r["y"], dtype=np.float32)
    y += np.asarray(b_out, dtype=np.float32)
    _CACHE["last_result"] = res
    return y


# revision 33
# speedup vs baseline: 1.0561x; 1.0561x over previous
"""Multi-head self-attention on 8 Trainium2 NeuronCores.

Sharding: tensor-parallel over heads (2 heads per core, both batch elements
on every core). Each core computes qkv projection / attention / its slice of
the output projection (rows of W_out for its heads), producing a partial
[B, N, D] output (bf16); the host sums the 8 partials and adds b_out.

Per-core dataflow (layouts chosen so no engine ever needs a cross-partition
shift except via DMA):
  - host supplies x^T [B, D, N] so the QKV projection can run directly
    (contraction dim on partitions for both operands)
  - QKV^T = Wsel^T @ x^T -> Q^T, K^T, V^T, each [128=2*64 head rows, N]
  - V^T is PE-transposed back to V [k, e] chunks, with a LEADING ones column
    per head so the P@V matmul also produces the softmax row-sums, landing
    on psum partition 0 where the gpsimd broadcast can read them directly
  - S^T = K^T(head)^T-block @ Q^T (contraction = head dim 64). The two
    heads' S matmuls write one shared [128, 2, 512] psum tile and carry
    disjoint PE row-tiles (auto tile_position (0,0)/(64,0)), so the
    hardware runs them concurrently and one exp covers both heads.
  - P^T = exp(S^T / sqrt(dp)) fused in the PSUM->SBUF evacuation on ScalarE
    (no max subtraction: scores are ~N(0,1), exp is safe in fp32)
  - O^T_aug = [1|V]^T-block @ P^T -> row 0 = softmax denominator s, rows
    1..64 = unnormalized O^T; normalize via fast-approx reciprocal + gpsimd
    partition_broadcast + multiply, DMA-shift into the combined O^T tile
  - y_partial = O^T-block^T @ W_out_slice

QCH=512 keeps every psum user at 1-2 banks: 4 banks S double-buffer +
2 banks PV accumulators + 2 banks for filler work (projection blocks and
the deferred QKV pieces), so fillers never contend with the S ring.
Deferred prep is deadline-scheduled into the attention windows.
"""

import numpy as np
import ml_dtypes

B, N, D, H, DP = 2, 2048, 1024, 16, 64
SCALE = float(DP) ** 0.5
NCORES = 8
HC = H // NCORES            # heads per core = 2
E = HC * DP                 # per-core head-dim total = 128
QCH = 512                   # q columns handled per attention chunk
NQ = N // QCH               # 4
KB = N // 128               # 16 k blocks
DC = D // 128               # 8 contraction chunks for the qkv projection

BF16 = ml_dtypes.bfloat16

_CACHE = {}


def _build_bass(with_bias=False):
    import concourse.bass as bass
    import concourse.mybir as mybir
    import concourse.tile as tile
    from concourse import bacc
    from concourse.masks import make_identity

    MM_DT = mybir.dt.bfloat16    # matmul input dtype
    P_DT = mybir.dt.bfloat16     # exp(S^T) storage dtype
    F32 = mybir.dt.float32

    # nonzero b_qkv is handled by an extra contraction chunk whose x^T rows
    # are [ones, 0...] and whose weight rows carry the bias (bias as matmul)
    DCX = DC + (1 if with_bias else 0)
    VAW = 130  # VA free width: 2 heads x [ones | V(64)]
    RING = 4   # P^T ring depth (PV runs at lag 2)
    nc = bacc.Bacc(None, target_bir_lowering=False)
    xt = nc.dram_tensor("xt", [B, DCX * 128, N], MM_DT, kind="ExternalInput")[:]
    wsel = nc.dram_tensor("wsel", [DCX * 128, 3 * E], MM_DT, kind="ExternalInput")[:]
    wout = nc.dram_tensor("wout", [E, D], MM_DT, kind="ExternalInput")[:]
    # bf16 partials halve the output DMA; the host sums in fp32
    y = nc.dram_tensor("y", [B, N, D], MM_DT, kind="ExternalOutput")[:]

    with tile.TileContext(nc) as tc:
        with (
            tc.tile_pool(name="consts", bufs=1) as consts,
            tc.tile_pool(name="xtp", bufs=2) as xtp,
            tc.tile_pool(name="ptp", bufs=4) as ptp,
            tc.tile_pool(name="qkvp", bufs=2) as qkvp,
            tc.tile_pool(name="vap", bufs=2) as vap,
            tc.tile_pool(name="otp", bufs=2) as otp,
            tc.tile_pool(name="evacp", bufs=2) as evacp,
            tc.tile_pool(name="normp", bufs=2) as normp,
            # 8 psum banks total: paired-S 2x[128,2,512]f32 (4) +
            # pv accumulators 2x[65,512] (2) + filler scratch 2x[128,512] (2)
            tc.tile_pool(name="ps_s", bufs=2, space="PSUM") as ps_s,
            tc.tile_pool(name="ps_g", bufs=2, space="PSUM") as ps_g,
            tc.tile_pool(name="ps_y", bufs=2, space="PSUM") as ps_y,
        ):
            # DMA issue order matters: the Sync engine issues serially and the
            # first QKV matmul waits on xt[b=0] chunk 0 + WS, so those go
            # first; nk-halved transfers let the nk=0 projections (all the
            # attention start needs) complete in half the bytes
            XTs = []
            for b in range(B):
                XTs.append(xtp.tile([128, DCX, N], MM_DT, tag="xt", name="xt"))
            WS = consts.tile([128, DCX, 3 * E], MM_DT)
            wsr = wsel.rearrange("(dc p) e -> p dc e", p=128)
            xtb0 = xt[0].rearrange("(dc p) n -> p dc n", p=128)
            for dc in range(DCX):
                nc.sync.dma_start(out=XTs[0][:, dc, 0:1024], in_=xtb0[:, dc, 0:1024])
                nc.sync.dma_start(out=WS[:, dc, :], in_=wsr[:, dc, :])
            xtb1 = xt[1].rearrange("(dc p) n -> p dc n", p=128)
            for dc in range(DCX):
                nc.sync.dma_start(out=XTs[0][:, dc, 1024:], in_=xtb0[:, dc, 1024:])
            for nk in range(2):
                for dc in range(DCX):
                    nc.sync.dma_start(
                        out=XTs[1][:, dc, nk * 1024 : (nk + 1) * 1024],
                        in_=xtb1[:, dc, nk * 1024 : (nk + 1) * 1024],
                    )
            WOUT = consts.tile([128, D], MM_DT)
            nc.sync.dma_start(out=WOUT, in_=wout)
            IDENT = consts.tile([128, 128], MM_DT)
            make_identity(nc, IDENT)
            WARM = consts.tile([1, 1], F32)
            nc.vector.memset(WARM, 0.0)
            nc.scalar.activation(
                out=WARM, in_=WARM, func=mybir.ActivationFunctionType.Exp
            )
            # p-state warmup: the PE clock ramps 0.65 -> 1.2 -> 2.4 GHz after
            # ~3us of continuous busy. Junk transposes while the first x^T /
            # W chunks stream in mean the real matmuls start at full clock.
            WARMPS = ps_g.tile([128, 128], MM_DT, tag="g", name="warm_ps")
            for _ in range(24):
                nc.tensor.transpose(WARMPS, IDENT, IDENT)

            QKVTs, VAs = [], []
            fillers = []  # deferred projection sub-blocks (no deadline)
            for b in range(B):
                QKVTs.append(
                    [
                        qkvp.tile([128, N], MM_DT, tag=f"qkv{eb}", name=f"qkv{eb}")
                        for eb in range(3)
                    ]
                )
                # V chunks with a LEADING ones column: [1 | V_h0(64) | 1 | V_h1]
                VA = vap.tile([128, KB, VAW], MM_DT, tag="va", name="va")
                nc.gpsimd.memset(VA[:, :, 0:1], 1.0)
                nc.gpsimd.memset(VA[:, :, VAW // 2 : VAW // 2 + 1], 1.0)
                VAs.append(VA)

            def emit_qkv_half(b2, eb, nk, hf, pool):
                # half of a projection block: one 512-col chunk of QKV^T
                tag = "g" if pool is ps_g else "q"
                ps = pool.tile([128, 512], F32, tag=tag, name="ps_qkv")
                c0 = nk * 1024 + hf * 512
                for dc in range(DCX):
                    nc.tensor.matmul(
                        ps,
                        lhsT=WS[:, dc, eb * 128 : (eb + 1) * 128],
                        rhs=XTs[b2][:, dc, c0 : c0 + 512],
                        start=(dc == 0),
                        stop=(dc == DCX - 1),
                    )
                nc.vector.tensor_copy(
                    out=QKVTs[b2][eb][:, c0 : c0 + 512], in_=ps
                )

            def emit_vtrans(b2, kc, pool):
                tag = "g" if pool is ps_g else "q"
                pst = pool.tile([128, 128], MM_DT, tag=tag, name="ps_vt")
                VT2 = QKVTs[b2][2]
                VA2 = VAs[b2]
                nc.tensor.transpose(
                    pst, VT2[:, kc * 128 : (kc + 1) * 128], IDENT
                )
                nc.vector.tensor_copy(out=VA2[:, kc, 1 : 1 + DP], in_=pst[:, 0:DP])
                nc.vector.tensor_copy(
                    out=VA2[:, kc, VAW // 2 + 1 : VAW // 2 + 1 + DP],
                    in_=pst[:, DP : 2 * DP],
                )

            # ---- deferred-prep schedule. Window index W counts kc windows
            # globally (16 per qh, 64 per batch). Each prep item carries the
            # last window index at which it may be emitted (one before its
            # first reader); pops happen at the top of each window, so a
            # deadline of W is safe for readers inside window W+1.
            def deadlines(base, items):
                return [(base + dl, it) for dl, it in items]

            prep = []
            for b2 in range(B):
                base = 64 * b2
                items = []
                for nk in range(2):
                    for hf in range(2):
                        qi = 2 * nk + hf  # qh index using these Q cols
                        items.append(
                            (16 * qi - 2,
                             lambda b2=b2, nk=nk, hf=hf: emit_qkv_half(
                                 b2, 0, nk, hf, ps_y))
                        )
                        kf = 8 * nk + 4 * hf  # first k-block in these cols
                        items.append(
                            (kf - 2,
                             lambda b2=b2, nk=nk, hf=hf: emit_qkv_half(
                                 b2, 1, nk, hf, ps_y))
                        )
                        items.append(
                            (kf - 1,
                             lambda b2=b2, nk=nk, hf=hf: emit_qkv_half(
                                 b2, 2, nk, hf, ps_y))
                        )
                for kc in range(KB):
                    # first PV reader of VA[kc] is emitted in window kc+3
                    items.append(
                        (kc - 1,
                         lambda b2=b2, kc=kc: emit_vtrans(b2, kc, ps_y))
                    )
                prep.extend(deadlines(base, items))
            prep.sort(key=lambda it: it[0])

            # b=0 items that would be due before the attention loop begins
            # run serially now (ps_g is free until the first pv allocation)
            while prep and prep[0][0] < 1:
                _, it = prep.pop(0)
                it()

            def emit_proj_block(spec, pool=None):
                b2, OT2, nb = spec
                pool = ps_y if pool is None else pool
                tag = "g" if pool is ps_g else "q"
                ysb = evacp.tile([128, D], MM_DT, tag="y", name="ysb", bufs=4)
                for dc2 in range(D // 512):
                    py = pool.tile([128, 512], F32, tag=tag, name="py")
                    nc.tensor.matmul(
                        py,
                        lhsT=OT2[:, nb * 128 : (nb + 1) * 128],
                        rhs=WOUT[:, dc2 * 512 : (dc2 + 1) * 512],
                        start=True,
                        stop=True,
                    )
                    nc.vector.tensor_copy(
                        out=ysb[:, dc2 * 512 : (dc2 + 1) * 512], in_=py
                    )
                nc.sync.dma_start(
                    out=y[b2, nb * 128 : (nb + 1) * 128, :], in_=ysb
                )

            # ---- phase 2: attention
            W = 0  # global window counter
            for b in range(B):
                QT, KT, VT = QKVTs[b]
                VA = VAs[b]
                OT = otp.tile([128, N], MM_DT, tag="ot", name="ot")
                for qh in range(NQ):
                    PT = ptp.tile(
                        [128, RING, HC, 512], P_DT, tag="pt", name="pt"
                    )
                    # one 1-bank PV accumulator per head, held across kc
                    pvs = [
                        ps_g.tile([DP + 1, 512], F32, tag="g", name=f"pv{h}")
                        for h in range(HC)
                    ]

                    def pv_mms(kc):
                        for h in range(HC):
                            nc.tensor.matmul(
                                pvs[h],
                                lhsT=VA[
                                    :, kc,
                                    h * (VAW // 2) : h * (VAW // 2) + DP + 1,
                                ],
                                rhs=PT[:, kc % RING, h, :],
                                start=(kc == 0),
                                stop=(kc == KB - 1),
                            )

                    for kc in range(KB):
                        # PE order per window: PV(kc-2), fillers, S(kc) — the
                        # already-runnable work absorbs the wait for exp(kc-2)
                        # to release the S psum tile
                        if kc >= 2:
                            pv_mms(kc - 2)
                        # mandatory deadline pops, then one opportunistic pop
                        popped = False
                        while prep and prep[0][0] <= W:
                            prep.pop(0)[1]()
                            popped = True
                        if not popped and prep and kc >= 1:
                            prep.pop(0)[1]()
                        elif not popped and fillers and kc >= 2:
                            fillers.pop(0)()
                        # both heads' S matmuls share one psum tile and
                        # disjoint PE row-tiles -> hardware runs them
                        # concurrently; one exp evacuates both
                        ps2 = ps_s.tile([128, HC, 512], F32, tag="s", name="s2")
                        q0 = qh * QCH
                        for h in range(HC):
                            nc.tensor.matmul(
                                ps2[:, h, :],
                                lhsT=KT[
                                    h * DP : (h + 1) * DP,
                                    kc * 128 : (kc + 1) * 128,
                                ],
                                rhs=QT[h * DP : (h + 1) * DP, q0 : q0 + 512],
                                start=True,
                                stop=True,
                            )
                        nc.scalar.activation(
                            out=PT[:, kc % RING, :, :],
                            in_=ps2,
                            func=mybir.ActivationFunctionType.Exp,
                            scale=1.0 / SCALE,
                        )
                        W += 1
                    while prep and prep[0][0] <= W:
                        prep.pop(0)[1]()
                    pv_mms(KB - 2)
                    pv_mms(KB - 1)

                    # normalize: denominator row is psum partition 0 (leading
                    # ones column). Evacuate pv on the scalar engine (slack at
                    # every boundary; frees the psum bank immediately), then
                    # fast-approx reciprocal, gpsimd broadcast, multiply
                    # (rows 0..64 for base-partition alignment; row 0 unused),
                    # DMA-shift into O^T.
                    # stage-interleaved across the two heads so the in-order
                    # DVE stream (copies, recips, muls) never stalls waiting
                    # for a gpsimd broadcast — the broadcasts run while the
                    # other head's DVE work proceeds
                    ocps, rts, bcs = [], [], []
                    for h in range(HC):
                        ocp = normp.tile([DP + 1, QCH], F32, tag="ocp", name="ocp")
                        nc.vector.tensor_copy(out=ocp, in_=pvs[h])
                        ocps.append(ocp)
                    for h in range(HC):
                        rt = normp.tile([1, QCH], F32, tag="rt", name="rt")
                        nc.vector.reciprocal_approx_fast(
                            out=rt, in_=ocps[h][0:1, :]
                        )
                        rts.append(rt)
                    for h in range(HC):
                        bc = normp.tile([DP + 1, QCH], F32, tag="bc", name="bc")
                        nc.gpsimd.partition_broadcast(bc, rts[h])
                        bcs.append(bc)
                    for h in range(HC):
                        ots = normp.tile([DP + 1, QCH], MM_DT, tag="ots", name="ots")
                        nc.vector.tensor_mul(out=ots, in0=ocps[h], in1=bcs[h])
                        nc.sync.dma_start(
                            out=OT[h * DP : (h + 1) * DP, qh * QCH : (qh + 1) * QCH],
                            in_=ots[1 : DP + 1, :],
                        )

                    # queue this qh's projection blocks as fillers (their
                    # norm-chain inputs are ready well before they are popped)
                    for nb in range(qh * QCH // 128, (qh + 1) * QCH // 128):
                        fillers.append(
                            (lambda pool=None, s=(b, OT, nb): emit_proj_block(s, pool))
                        )

            # drain remaining fillers; pv accumulators are dead, so alternate
            # psum pools to keep 4 blocks in flight
            di = 0
            while fillers:
                fillers.pop(0)(ps_g if di % 2 else ps_y)
                di += 1
    nc.finalize()
    return nc


def _get_bass(with_bias=False):
    key = f"nc{int(with_bias)}"
    if key not in _CACHE:
        _CACHE[key] = _build_bass(with_bias)
    return _CACHE[key]


def _make_in_maps(x, W_qkv, b_qkv, W_out):
    """Shard the full inputs into the 8 per-core input dicts."""
    x = np.asarray(x, dtype=np.float32)
    W_qkv = np.asarray(W_qkv, dtype=np.float32)
    b_qkv = np.asarray(b_qkv, dtype=np.float32)
    W_out = np.asarray(W_out, dtype=np.float32)

    with_bias = bool(np.any(b_qkv))
    # x^T per batch, shared by all cores (+ optional bias chunk rows)
    xtt = x.transpose(0, 2, 1)
    if with_bias:
        aug = np.zeros((B, 128, N), dtype=np.float32)
        aug[:, 0, :] = 1.0
        xtt = np.concatenate([xtt, aug], axis=1)
    xt = np.ascontiguousarray(xtt).astype(BF16)

    in_maps = []
    for c in range(NCORES):
        heads = [HC * c + i for i in range(HC)]
        # W_qkv columns: head h occupies cols [h*3*DP, (h+1)*3*DP) as [q|k|v]
        qcols = [W_qkv[:, h * 3 * DP : h * 3 * DP + DP] for h in heads]
        kcols = [W_qkv[:, h * 3 * DP + DP : h * 3 * DP + 2 * DP] for h in heads]
        vcols = [W_qkv[:, h * 3 * DP + 2 * DP : h * 3 * DP + 3 * DP] for h in heads]
        wsel = np.concatenate(qcols + kcols + vcols, axis=1)  # [D, 3*E]
        if with_bias:
            bq = [b_qkv[h * 3 * DP : h * 3 * DP + DP] for h in heads]
            bk = [b_qkv[h * 3 * DP + DP : h * 3 * DP + 2 * DP] for h in heads]
            bv = [b_qkv[h * 3 * DP + 2 * DP : h * 3 * DP + 3 * DP] for h in heads]
            brow = np.concatenate(bq + bk + bv)  # [3*E]
            baug = np.zeros((128, 3 * E), dtype=np.float32)
            baug[0, :] = brow
            wsel = np.concatenate([wsel, baug], axis=0)
        woutc = np.concatenate(
            [W_out[h * DP : (h + 1) * DP, :] for h in heads], axis=0
        )  # [E, D]
        in_maps.append(
            {
                "xt": xt,
                "wsel": np.ascontiguousarray(wsel).astype(BF16),
                "wout": np.ascontiguousarray(woutc).astype(BF16),
            }
        )
    return in_maps, with_bias


def _run(in_maps, with_bias=False, trace=False):
    from concourse import bass_utils

    nc = _get_bass(with_bias)
    return bass_utils.run_bass_kernel_spmd(
        nc, in_maps, core_ids=list(range(NCORES)), trace=trace
    )


def kernel(x, W_qkv, b_qkv, W_out, b_out, _trace=False):
    in_maps, with_bias = _make_in_maps(x, W_qkv, b_qkv, W_out)
    res = _run(in_maps, with_bias=with_bias, trace=_trace)
    y = np.zeros((B, N, D), dtype=np.float32)
    for r in res.results:
        y += np.asarray(r["y"], dtype=np.float32)
    y += np.asarray(b_out, dtype=np.float32)
    _CACHE["last_result"] = res
    return y


# revision 36
# speedup vs baseline: 1.0631x; 1.0067x over previous
"""Multi-head self-attention on 8 Trainium2 NeuronCores.

Sharding: tensor-parallel over heads (2 heads per core, both batch elements
on every core). Each core computes qkv projection / attention / its slice of
the output projection (rows of W_out for its heads), producing a partial
[B, N, D] output (bf16); the host sums the 8 partials and adds b_out.

Per-core dataflow (layouts chosen so no engine ever needs a cross-partition
shift except via DMA):
  - host supplies x^T [B, D, N] so the QKV projection can run directly
    (contraction dim on partitions for both operands)
  - QKV^T = Wsel^T @ x^T -> Q^T, K^T, V^T, each [128=2*64 head rows, N]
  - V^T is PE-transposed back to V [k, e] chunks, with a LEADING ones column
    per head so the P@V matmul also produces the softmax row-sums, landing
    on psum partition 0 where the gpsimd broadcast can read them directly
  - S^T = K^T(head)^T-block @ Q^T (contraction = head dim 64). The two
    heads' S matmuls write one shared [128, 2, 512] psum tile and carry
    disjoint PE row-tiles (auto tile_position (0,0)/(64,0)), so the
    hardware runs them concurrently and one exp covers both heads.
  - P^T = exp(S^T / sqrt(dp)) fused in the PSUM->SBUF evacuation on ScalarE
    (no max subtraction: scores are ~N(0,1), exp is safe in fp32)
  - O^T_aug = [1|V]^T-block @ P^T -> row 0 = softmax denominator s, rows
    1..64 = unnormalized O^T; normalize via fast-approx reciprocal + gpsimd
    partition_broadcast + multiply, DMA-shift into the combined O^T tile
  - y_partial = O^T-block^T @ W_out_slice

QCH=512 keeps every psum user at 1-2 banks: 4 banks S double-buffer +
2 banks PV accumulators + 2 banks for filler work (projection blocks and
the deferred QKV pieces), so fillers never contend with the S ring.
Deferred prep is deadline-scheduled into the attention windows.
"""

import numpy as np
import ml_dtypes

B, N, D, H, DP = 2, 2048, 1024, 16, 64
SCALE = float(DP) ** 0.5
NCORES = 8
HC = H // NCORES            # heads per core = 2
E = HC * DP                 # per-core head-dim total = 128
QCH = 512                   # q columns handled per attention chunk
NQ = N // QCH               # 4
KB = N // 128               # 16 k blocks
DC = D // 128               # 8 contraction chunks for the qkv projection

BF16 = ml_dtypes.bfloat16

_CACHE = {}


def _build_bass(with_bias=False):
    import concourse.bass as bass
    import concourse.mybir as mybir
    import concourse.tile as tile
    from concourse import bacc
    from concourse.masks import make_identity

    MM_DT = mybir.dt.bfloat16    # matmul input dtype
    P_DT = mybir.dt.bfloat16     # exp(S^T) storage dtype
    F32 = mybir.dt.float32

    # nonzero b_qkv is handled by an extra contraction chunk whose x^T rows
    # are [ones, 0...] and whose weight rows carry the bias (bias as matmul)
    DCX = DC + (1 if with_bias else 0)
    VAW = 130  # VA free width: 2 heads x [ones | V(64)]
    RING = 4   # P^T ring depth (PV runs at lag 2)
    nc = bacc.Bacc(None, target_bir_lowering=False)
    xt = nc.dram_tensor("xt", [B, DCX * 128, N], MM_DT, kind="ExternalInput")[:]
    wsel = nc.dram_tensor("wsel", [DCX * 128, 3 * E], MM_DT, kind="ExternalInput")[:]
    wout = nc.dram_tensor("wout", [E, D], MM_DT, kind="ExternalInput")[:]
    # bf16 partials halve the output DMA; the host sums in fp32
    y = nc.dram_tensor("y", [B, N, D], MM_DT, kind="ExternalOutput")[:]

    with tile.TileContext(nc) as tc:
        with (
            tc.tile_pool(name="consts", bufs=1) as consts,
            tc.tile_pool(name="xtp", bufs=2) as xtp,
            tc.tile_pool(name="ptp", bufs=4) as ptp,
            tc.tile_pool(name="qkvp", bufs=2) as qkvp,
            tc.tile_pool(name="vap", bufs=2) as vap,
            tc.tile_pool(name="otp", bufs=2) as otp,
            tc.tile_pool(name="evacp", bufs=2) as evacp,
            tc.tile_pool(name="normp", bufs=2) as normp,
            # 8 psum banks total: paired-S 2x[128,2,512]f32 (4) +
            # pv accumulators 2x[65,512] (2) + filler scratch 2x[128,512] (2)
            tc.tile_pool(name="ps_s", bufs=2, space="PSUM") as ps_s,
            tc.tile_pool(name="ps_g", bufs=2, space="PSUM") as ps_g,
            tc.tile_pool(name="ps_y", bufs=2, space="PSUM") as ps_y,
        ):
            # DMA issue order matters: the Sync engine issues serially and the
            # first QKV matmul waits on xt[b=0] chunk 0 + WS, so those go
            # first; nk-halved transfers let the nk=0 projections (all the
            # attention start needs) complete in half the bytes
            XTs = []
            for b in range(B):
                XTs.append(xtp.tile([128, DCX, N], MM_DT, tag="xt", name="xt"))
            WS = consts.tile([128, DCX, 3 * E], MM_DT)
            wsr = wsel.rearrange("(dc p) e -> p dc e", p=128)
            xtb0 = xt[0].rearrange("(dc p) n -> p dc n", p=128)
            for dc in range(DCX):
                nc.sync.dma_start(out=XTs[0][:, dc, 0:1024], in_=xtb0[:, dc, 0:1024])
                nc.sync.dma_start(out=WS[:, dc, :], in_=wsr[:, dc, :])
            xtb1 = xt[1].rearrange("(dc p) n -> p dc n", p=128)
            for dc in range(DCX):
                nc.sync.dma_start(out=XTs[0][:, dc, 1024:], in_=xtb0[:, dc, 1024:])
            for nk in range(2):
                for dc in range(DCX):
                    nc.sync.dma_start(
                        out=XTs[1][:, dc, nk * 1024 : (nk + 1) * 1024],
                        in_=xtb1[:, dc, nk * 1024 : (nk + 1) * 1024],
                    )
            WOUT = consts.tile([128, D], MM_DT)
            nc.sync.dma_start(out=WOUT, in_=wout)
            IDENT = consts.tile([128, 128], MM_DT)
            make_identity(nc, IDENT)
            WARM = consts.tile([1, 1], F32)
            nc.vector.memset(WARM, 0.0)
            nc.scalar.activation(
                out=WARM, in_=WARM, func=mybir.ActivationFunctionType.Exp
            )
            # p-state warmup: the PE clock ramps 0.65 -> 1.2 -> 2.4 GHz after
            # ~3us of continuous busy. Junk transposes while the first x^T /
            # W chunks stream in mean the real matmuls start at full clock.
            WARMPS = ps_g.tile([128, 128], MM_DT, tag="g", name="warm_ps")
            for _ in range(24):
                nc.tensor.transpose(WARMPS, IDENT, IDENT)

            QKVTs, VAs = [], []
            fillers = []  # deferred projection sub-blocks (no deadline)
            for b in range(B):
                QKVTs.append(
                    [
                        qkvp.tile([128, N], MM_DT, tag=f"qkv{eb}", name=f"qkv{eb}")
                        for eb in range(3)
                    ]
                )
                # V chunks with a LEADING ones column: [1 | V_h0(64) | 1 | V_h1]
                VA = vap.tile([128, KB, VAW], MM_DT, tag="va", name="va")
                nc.gpsimd.memset(VA[:, :, 0:1], 1.0)
                nc.gpsimd.memset(VA[:, :, VAW // 2 : VAW // 2 + 1], 1.0)
                VAs.append(VA)

            def emit_qkv_half(b2, eb, nk, hf, pool):
                # half of a projection block: one 512-col chunk of QKV^T
                tag = "g" if pool is ps_g else "q"
                ps = pool.tile([128, 512], F32, tag=tag, name="ps_qkv")
                c0 = nk * 1024 + hf * 512
                for dc in range(DCX):
                    nc.tensor.matmul(
                        ps,
                        lhsT=WS[:, dc, eb * 128 : (eb + 1) * 128],
                        rhs=XTs[b2][:, dc, c0 : c0 + 512],
                        start=(dc == 0),
                        stop=(dc == DCX - 1),
                    )
                nc.vector.tensor_copy(
                    out=QKVTs[b2][eb][:, c0 : c0 + 512], in_=ps
                )

            def emit_vtrans(b2, kc, pool):
                tag = "g" if pool is ps_g else "q"
                pst = pool.tile([128, 128], MM_DT, tag=tag, name="ps_vt")
                VT2 = QKVTs[b2][2]
                VA2 = VAs[b2]
                nc.tensor.transpose(
                    pst, VT2[:, kc * 128 : (kc + 1) * 128], IDENT
                )
                nc.vector.tensor_copy(out=VA2[:, kc, 1 : 1 + DP], in_=pst[:, 0:DP])
                nc.vector.tensor_copy(
                    out=VA2[:, kc, VAW // 2 + 1 : VAW // 2 + 1 + DP],
                    in_=pst[:, DP : 2 * DP],
                )

            # ---- deferred-prep schedule. Window index W counts kc windows
            # globally (16 per qh, 64 per batch). Each prep item carries the
            # last window index at which it may be emitted (one before its
            # first reader); pops happen at the top of each window, so a
            # deadline of W is safe for readers inside window W+1.
            def deadlines(base, items):
                return [(base + dl, it) for dl, it in items]

            prep = []
            for b2 in range(B):
                base = 64 * b2
                items = []
                for nk in range(2):
                    for hf in range(2):
                        qi = 2 * nk + hf  # qh index using these Q cols
                        items.append(
                            (16 * qi - 2,
                             lambda b2=b2, nk=nk, hf=hf: emit_qkv_half(
                                 b2, 0, nk, hf, ps_y))
                        )
                        kf = 8 * nk + 4 * hf  # first k-block in these cols
                        items.append(
                            (kf - 2,
                             lambda b2=b2, nk=nk, hf=hf: emit_qkv_half(
                                 b2, 1, nk, hf, ps_y))
                        )
                        items.append(
                            (kf - 1,
                             lambda b2=b2, nk=nk, hf=hf: emit_qkv_half(
                                 b2, 2, nk, hf, ps_y))
                        )
                for kc in range(KB):
                    # first PV reader of VA[kc] is emitted in window kc+2
                    items.append(
                        (kc,
                         lambda b2=b2, kc=kc: emit_vtrans(b2, kc, ps_y))
                    )
                prep.extend(deadlines(base, items))
            prep.sort(key=lambda it: it[0])

            # b=0 items that would be due before the attention loop begins
            # run serially now (ps_g is free until the first pv allocation)
            while prep and prep[0][0] < 1:
                _, it = prep.pop(0)
                it()

            def emit_proj_block(spec, pool=None):
                b2, OT2, nb = spec
                pool = ps_y if pool is None else pool
                tag = "g" if pool is ps_g else "q"
                ysb = evacp.tile([128, D], MM_DT, tag="y", name="ysb", bufs=4)
                for dc2 in range(D // 512):
                    py = pool.tile([128, 512], F32, tag=tag, name="py")
                    nc.tensor.matmul(
                        py,
                        lhsT=OT2[:, nb * 128 : (nb + 1) * 128],
                        rhs=WOUT[:, dc2 * 512 : (dc2 + 1) * 512],
                        start=True,
                        stop=True,
                    )
                    nc.vector.tensor_copy(
                        out=ysb[:, dc2 * 512 : (dc2 + 1) * 512], in_=py
                    )
                nc.sync.dma_start(
                    out=y[b2, nb * 128 : (nb + 1) * 128, :], in_=ysb
                )

            # ---- phase 2: attention
            W = 0  # global window counter
            for b in range(B):
                QT, KT, VT = QKVTs[b]
                VA = VAs[b]
                OT = otp.tile([128, N], MM_DT, tag="ot", name="ot")
                for qh in range(NQ):
                    PT = ptp.tile(
                        [128, RING, HC, 512], P_DT, tag="pt", name="pt"
                    )
                    # one 1-bank PV accumulator per head, held across kc
                    pvs = [
                        ps_g.tile([DP + 1, 512], F32, tag="g", name=f"pv{h}")
                        for h in range(HC)
                    ]

                    def pv_mms(kc):
                        for h in range(HC):
                            nc.tensor.matmul(
                                pvs[h],
                                lhsT=VA[
                                    :, kc,
                                    h * (VAW // 2) : h * (VAW // 2) + DP + 1,
                                ],
                                rhs=PT[:, kc % RING, h, :],
                                start=(kc == 0),
                                stop=(kc == KB - 1),
                            )

                    for kc in range(KB):
                        # PE order per window: PV(kc-2), fillers, S(kc) — the
                        # already-runnable work absorbs the wait for exp(kc-2)
                        # to release the S psum tile
                        if kc >= 2:
                            pv_mms(kc - 2)
                        # mandatory deadline pops, then one opportunistic pop
                        popped = False
                        while prep and prep[0][0] <= W:
                            prep.pop(0)[1]()
                            popped = True
                        if not popped and prep and kc >= 1:
                            prep.pop(0)[1]()
                        elif not popped and fillers and kc >= 2:
                            fillers.pop(0)()
                        # both heads' S matmuls share one psum tile and
                        # disjoint PE row-tiles -> hardware runs them
                        # concurrently; one exp evacuates both
                        ps2 = ps_s.tile([128, HC, 512], F32, tag="s", name="s2")
                        q0 = qh * QCH
                        for h in range(HC):
                            nc.tensor.matmul(
                                ps2[:, h, :],
                                lhsT=KT[
                                    h * DP : (h + 1) * DP,
                                    kc * 128 : (kc + 1) * 128,
                                ],
                                rhs=QT[h * DP : (h + 1) * DP, q0 : q0 + 512],
                                start=True,
                                stop=True,
                            )
                        nc.scalar.activation(
                            out=PT[:, kc % RING, :, :],
                            in_=ps2,
                            func=mybir.ActivationFunctionType.Exp,
                            scale=1.0 / SCALE,
                        )
                        W += 1
                    pv_mms(KB - 2)
                    pv_mms(KB - 1)

                    # normalize: denominator row is psum partition 0 (leading
                    # ones column). Evacuate pv on the scalar engine (slack at
                    # every boundary; frees the psum bank immediately), then
                    # fast-approx reciprocal, gpsimd broadcast, multiply
                    # (rows 0..64 for base-partition alignment; row 0 unused),
                    # DMA-shift into O^T.
                    for h in range(HC):
                        pv = pvs[h]
                        ocp = normp.tile([DP + 1, QCH], F32, tag="ocp", name="ocp")
                        nc.vector.tensor_copy(out=ocp, in_=pv)
                        rt = normp.tile([1, QCH], F32, tag="rt", name="rt")
                        nc.vector.reciprocal_approx_fast(out=rt, in_=ocp[0:1, :])
                        bc = normp.tile([DP + 1, QCH], F32, tag="bc", name="bc")
                        nc.gpsimd.partition_broadcast(bc, rt)
                        ots = normp.tile([DP + 1, QCH], MM_DT, tag="ots", name="ots")
                        nc.vector.tensor_mul(out=ots, in0=ocp, in1=bc)
                        nc.sync.dma_start(
                            out=OT[h * DP : (h + 1) * DP, qh * QCH : (qh + 1) * QCH],
                            in_=ots[1 : DP + 1, :],
                        )

                    # queue this qh's projection blocks as fillers (their
                    # norm-chain inputs are ready well before they are popped)
                    for nb in range(qh * QCH // 128, (qh + 1) * QCH // 128):
                        fillers.append(
                            (lambda pool=None, s=(b, OT, nb): emit_proj_block(s, pool))
                        )

            # drain remaining fillers; pv accumulators are dead, so alternate
            # psum pools to keep 4 blocks in flight
            di = 0
            while fillers:
                fillers.pop(0)(ps_g if di % 2 else ps_y)
                di += 1
    nc.finalize()
    return nc


def _get_bass(with_bias=False):
    key = f"nc{int(with_bias)}"
    if key not in _CACHE:
        _CACHE[key] = _build_bass(with_bias)
    return _CACHE[key]


def _make_in_maps(x, W_qkv, b_qkv, W_out):
    """Shard the full inputs into the 8 per-core input dicts."""
    x = np.asarray(x, dtype=np.float32)
    W_qkv = np.asarray(W_qkv, dtype=np.float32)
    b_qkv = np.asarray(b_qkv, dtype=np.float32)
    W_out = np.asarray(W_out, dtype=np.float32)

    with_bias = bool(np.any(b_qkv))
    # x^T per batch, shared by all cores (+ optional bias chunk rows)
    xtt = x.transpose(0, 2, 1)
    if with_bias:
        aug = np.zeros((B, 128, N), dtype=np.float32)
        aug[:, 0, :] = 1.0
        xtt = np.concatenate([xtt, aug], axis=1)
    xt = np.ascontiguousarray(xtt).astype(BF16)

    in_maps = []
    for c in range(NCORES):
        heads = [HC * c + i for i in range(HC)]
        # W_qkv columns: head h occupies cols [h*3*DP, (h+1)*3*DP) as [q|k|v]
        qcols = [W_qkv[:, h * 3 * DP : h * 3 * DP + DP] for h in heads]
        kcols = [W_qkv[:, h * 3 * DP + DP : h * 3 * DP + 2 * DP] for h in heads]
        vcols = [W_qkv[:, h * 3 * DP + 2 * DP : h * 3 * DP + 3 * DP] for h in heads]
        wsel = np.concatenate(qcols + kcols + vcols, axis=1)  # [D, 3*E]
        if with_bias:
            bq = [b_qkv[h * 3 * DP : h * 3 * DP + DP] for h in heads]
            bk = [b_qkv[h * 3 * DP + DP : h * 3 * DP + 2 * DP] for h in heads]
            bv = [b_qkv[h * 3 * DP + 2 * DP : h * 3 * DP + 3 * DP] for h in heads]
            brow = np.concatenate(bq + bk + bv)  # [3*E]
            baug = np.zeros((128, 3 * E), dtype=np.float32)
            baug[0, :] = brow
            wsel = np.concatenate([wsel, baug], axis=0)
        woutc = np.concatenate(
            [W_out[h * DP : (h + 1) * DP, :] for h in heads], axis=0
        )  # [E, D]
        in_maps.append(
            {
                "xt": xt,
                "wsel": np.ascontiguousarray(wsel).astype(BF16),
                "wout": np.ascontiguousarray(woutc).astype(BF16),
            }
        )
    return in_maps, with_bias


def _run(in_maps, with_bias=False, trace=False):
    from concourse import bass_utils

    nc = _get_bass(with_bias)
    return bass_utils.run_bass_kernel_spmd(
        nc, in_maps, core_ids=list(range(NCORES)), trace=trace
    )


def kernel(x, W_qkv, b_qkv, W_out, b_out, _trace=False):
    in_maps, with_bias = _make_in_maps(x, W_qkv, b_qkv, W_out)
    res = _run(in_maps, with_bias=with_bias, trace=_trace)
    y = np.zeros((B, N, D), dtype=np.float32)
    for r in res.results:
        y += np.asarray(r["y"], dtype=np.float32)
    y += np.asarray(b_out, dtype=np.float32)
    _CACHE["last_result"] = res
    return y


# revision 38
# speedup vs baseline: 1.1255x; 1.0587x over previous
"""Multi-head self-attention on 8 Trainium2 NeuronCores.

Sharding: tensor-parallel over heads (2 heads per core, both batch elements
on every core). Each core computes qkv projection / attention / its slice of
the output projection (rows of W_out for its heads), producing a partial
[B, N, D] output (bf16); the host sums the 8 partials and adds b_out.

Per-core dataflow (layouts chosen so no engine ever needs a cross-partition
shift except via DMA):
  - host supplies x^T [B, D, N] so the QKV projection can run directly
    (contraction dim on partitions for both operands)
  - QKV^T = Wsel^T @ x^T -> Q^T, K^T, V^T, each [128=2*64 head rows, N]
  - V^T is PE-transposed back to V [k, e] chunks, with a LEADING ones column
    per head so the P@V matmul also produces the softmax row-sums, landing
    on psum partition 0 where the gpsimd broadcast can read them directly
  - S^T = K^T(head)^T-block @ Q^T (contraction = head dim 64). The two
    heads' S matmuls write one shared [128, 2, 512] psum tile and carry
    disjoint PE row-tiles (auto tile_position (0,0)/(64,0)), so the
    hardware runs them concurrently and one exp covers both heads.
  - P^T = exp(S^T / sqrt(dp)) fused in the PSUM->SBUF evacuation on ScalarE
    (no max subtraction: scores are ~N(0,1), exp is safe in fp32)
  - O^T_aug = [1|V]^T-block @ P^T -> row 0 = softmax denominator s, rows
    1..64 = unnormalized O^T; normalize via fast-approx reciprocal + gpsimd
    partition_broadcast + multiply, DMA-shift into the combined O^T tile
  - y_partial = O^T-block^T @ W_out_slice

QCH=512 keeps every psum user at 1-2 banks: 4 banks S double-buffer +
2 banks PV accumulators + 2 banks for filler work (projection blocks and
the deferred QKV pieces), so fillers never contend with the S ring.
Deferred prep is deadline-scheduled into the attention windows.
"""

import numpy as np
import ml_dtypes

B, N, D, H, DP = 2, 2048, 1024, 16, 64
SCALE = float(DP) ** 0.5
NCORES = 8
HC = H // NCORES            # heads per core = 2
E = HC * DP                 # per-core head-dim total = 128
QCH = 512                   # q columns handled per attention chunk
NQ = N // QCH               # 4
KB = N // 128               # 16 k blocks
DC = D // 128               # 8 contraction chunks for the qkv projection

BF16 = ml_dtypes.bfloat16

_CACHE = {}


def _build_bass(with_bias=False):
    import concourse.bass as bass
    import concourse.mybir as mybir
    import concourse.tile as tile
    from concourse import bacc
    from concourse.masks import make_identity

    MM_DT = mybir.dt.bfloat16    # matmul input dtype
    P_DT = mybir.dt.bfloat16     # exp(S^T) storage dtype
    F32 = mybir.dt.float32

    # nonzero b_qkv is handled by an extra contraction chunk whose x^T rows
    # are [ones, 0...] and whose weight rows carry the bias (bias as matmul)
    DCX = DC + (1 if with_bias else 0)
    VAW = 130  # VA free width: 2 heads x [ones | V(64)]
    RING = 4   # P^T ring depth (PV runs at lag 2)
    nc = bacc.Bacc(None, target_bir_lowering=False)
    xt = nc.dram_tensor("xt", [B, DCX * 128, N], MM_DT, kind="ExternalInput")[:]
    wsel = nc.dram_tensor("wsel", [DCX * 128, 3 * E], MM_DT, kind="ExternalInput")[:]
    wout = nc.dram_tensor("wout", [E, D], MM_DT, kind="ExternalInput")[:]
    # bf16 partials halve the output DMA; the host sums in fp32
    y = nc.dram_tensor("y", [B, N, D], MM_DT, kind="ExternalOutput")[:]

    with tile.TileContext(nc) as tc:
        with (
            tc.tile_pool(name="consts", bufs=1) as consts,
            tc.tile_pool(name="xtp", bufs=2) as xtp,
            tc.tile_pool(name="ptp", bufs=4) as ptp,
            tc.tile_pool(name="qkvp", bufs=2) as qkvp,
            tc.tile_pool(name="vap", bufs=2) as vap,
            tc.tile_pool(name="otp", bufs=2) as otp,
            tc.tile_pool(name="evacp", bufs=2) as evacp,
            tc.tile_pool(name="normp", bufs=2) as normp,
            # 8 psum banks total: paired-S 2x[128,2,512]f32 (4) +
            # pv accumulators 2x[65,512] (2) + filler scratch 2x[128,512] (2)
            tc.tile_pool(name="ps_s", bufs=2, space="PSUM") as ps_s,
            tc.tile_pool(name="ps_g", bufs=2, space="PSUM") as ps_g,
            tc.tile_pool(name="ps_y", bufs=2, space="PSUM") as ps_y,
        ):
            # DMA issue order matters: the Sync engine issues serially and the
            # first QKV matmul waits on xt[b=0] chunk 0 + WS, so those go
            # first; nk-halved transfers let the nk=0 projections (all the
            # attention start needs) complete in half the bytes
            XTs = []
            for b in range(B):
                XTs.append(xtp.tile([128, DCX, N], MM_DT, tag="xt", name="xt"))
            WS = consts.tile([128, DCX, 3 * E], MM_DT)
            wsr = wsel.rearrange("(dc p) e -> p dc e", p=128)
            xtb0 = xt[0].rearrange("(dc p) n -> p dc n", p=128)
            # the serial prep consumes only columns 0-511, so quarter-sized
            # first chunks let it start ~3us earlier
            for dc in range(DCX):
                nc.sync.dma_start(out=XTs[0][:, dc, 0:512], in_=xtb0[:, dc, 0:512])
                nc.sync.dma_start(out=WS[:, dc, :], in_=wsr[:, dc, :])
            for dc in range(DCX):
                nc.sync.dma_start(
                    out=XTs[0][:, dc, 512:1024], in_=xtb0[:, dc, 512:1024]
                )
            xtb1 = xt[1].rearrange("(dc p) n -> p dc n", p=128)
            for dc in range(DCX):
                nc.sync.dma_start(out=XTs[0][:, dc, 1024:], in_=xtb0[:, dc, 1024:])
            for nk in range(2):
                for dc in range(DCX):
                    nc.sync.dma_start(
                        out=XTs[1][:, dc, nk * 1024 : (nk + 1) * 1024],
                        in_=xtb1[:, dc, nk * 1024 : (nk + 1) * 1024],
                    )
            WOUT = consts.tile([128, D], MM_DT)
            nc.sync.dma_start(out=WOUT, in_=wout)
            IDENT = consts.tile([128, 128], MM_DT)
            make_identity(nc, IDENT)
            WARM = consts.tile([1, 1], F32)
            nc.vector.memset(WARM, 0.0)
            nc.scalar.activation(
                out=WARM, in_=WARM, func=mybir.ActivationFunctionType.Exp
            )
            # p-state warmup: the PE clock ramps 0.65 -> 1.2 -> 2.4 GHz after
            # ~3us of continuous busy. Junk transposes while the first x^T /
            # W chunks stream in mean the real matmuls start at full clock.
            WARMPS = ps_g.tile([128, 128], MM_DT, tag="g", name="warm_ps")
            for _ in range(24):
                nc.tensor.transpose(WARMPS, IDENT, IDENT)

            QKVTs, VAs = [], []
            fillers = []  # deferred projection sub-blocks (no deadline)
            for b in range(B):
                QKVTs.append(
                    [
                        qkvp.tile([128, N], MM_DT, tag=f"qkv{eb}", name=f"qkv{eb}")
                        for eb in range(3)
                    ]
                )
                # V chunks with a LEADING ones column: [1 | V_h0(64) | 1 | V_h1]
                VA = vap.tile([128, KB, VAW], MM_DT, tag="va", name="va")
                nc.gpsimd.memset(VA[:, :, 0:1], 1.0)
                nc.gpsimd.memset(VA[:, :, VAW // 2 : VAW // 2 + 1], 1.0)
                VAs.append(VA)

            def emit_qkv_half(b2, eb, nk, hf, pool):
                # half of a projection block: one 512-col chunk of QKV^T
                tag = "g" if pool is ps_g else "q"
                ps = pool.tile([128, 512], F32, tag=tag, name="ps_qkv")
                c0 = nk * 1024 + hf * 512
                for dc in range(DCX):
                    nc.tensor.matmul(
                        ps,
                        lhsT=WS[:, dc, eb * 128 : (eb + 1) * 128],
                        rhs=XTs[b2][:, dc, c0 : c0 + 512],
                        start=(dc == 0),
                        stop=(dc == DCX - 1),
                    )
                nc.vector.tensor_copy(
                    out=QKVTs[b2][eb][:, c0 : c0 + 512], in_=ps
                )

            def emit_vtrans(b2, kc, pool):
                tag = "g" if pool is ps_g else "q"
                pst = pool.tile([128, 128], MM_DT, tag=tag, name="ps_vt")
                VT2 = QKVTs[b2][2]
                VA2 = VAs[b2]
                nc.tensor.transpose(
                    pst, VT2[:, kc * 128 : (kc + 1) * 128], IDENT
                )
                nc.vector.tensor_copy(out=VA2[:, kc, 1 : 1 + DP], in_=pst[:, 0:DP])
                nc.vector.tensor_copy(
                    out=VA2[:, kc, VAW // 2 + 1 : VAW // 2 + 1 + DP],
                    in_=pst[:, DP : 2 * DP],
                )

            # ---- deferred-prep schedule. Window index W counts kc windows
            # globally (16 per qh, 64 per batch). Each prep item carries the
            # last window index at which it may be emitted (one before its
            # first reader); pops happen at the top of each window, so a
            # deadline of W is safe for readers inside window W+1.
            def deadlines(base, items):
                return [(base + dl, it) for dl, it in items]

            prep = []
            for b2 in range(B):
                base = 64 * b2
                items = []
                for nk in range(2):
                    for hf in range(2):
                        qi = 2 * nk + hf  # qh index using these Q cols
                        items.append(
                            (16 * qi - 2,
                             lambda b2=b2, nk=nk, hf=hf: emit_qkv_half(
                                 b2, 0, nk, hf, ps_y))
                        )
                        kf = 8 * nk + 4 * hf  # first k-block in these cols
                        items.append(
                            (kf - 2,
                             lambda b2=b2, nk=nk, hf=hf: emit_qkv_half(
                                 b2, 1, nk, hf, ps_y))
                        )
                        items.append(
                            (kf - 1,
                             lambda b2=b2, nk=nk, hf=hf: emit_qkv_half(
                                 b2, 2, nk, hf, ps_y))
                        )
                for kc in range(KB):
                    # first PV reader of VA[kc] is emitted in window kc+2
                    items.append(
                        (kc,
                         lambda b2=b2, kc=kc: emit_vtrans(b2, kc, ps_y))
                    )
                prep.extend(deadlines(base, items))
            prep.sort(key=lambda it: it[0])

            # b=0 items that would be due before the attention loop begins
            # run serially now (ps_g is free until the first pv allocation)
            while prep and prep[0][0] < 1:
                _, it = prep.pop(0)
                it()

            def emit_proj_block(spec, pool=None):
                b2, OT2, nb = spec
                pool = ps_y if pool is None else pool
                tag = "g" if pool is ps_g else "q"
                ysb = evacp.tile([128, D], MM_DT, tag="y", name="ysb", bufs=4)
                for dc2 in range(D // 512):
                    py = pool.tile([128, 512], F32, tag=tag, name="py")
                    nc.tensor.matmul(
                        py,
                        lhsT=OT2[:, nb * 128 : (nb + 1) * 128],
                        rhs=WOUT[:, dc2 * 512 : (dc2 + 1) * 512],
                        start=True,
                        stop=True,
                    )
                    nc.vector.tensor_copy(
                        out=ysb[:, dc2 * 512 : (dc2 + 1) * 512], in_=py
                    )
                nc.sync.dma_start(
                    out=y[b2, nb * 128 : (nb + 1) * 128, :], in_=ysb
                )

            # ---- phase 2: attention
            W = 0  # global window counter
            for b in range(B):
                QT, KT, VT = QKVTs[b]
                VA = VAs[b]
                OT = otp.tile([128, N], MM_DT, tag="ot", name="ot")
                for qh in range(NQ):
                    PT = ptp.tile(
                        [128, RING, HC, 512], P_DT, tag="pt", name="pt"
                    )
                    # one 1-bank PV accumulator per head, held across kc
                    pvs = [
                        ps_g.tile([DP + 1, 512], F32, tag="g", name=f"pv{h}")
                        for h in range(HC)
                    ]

                    def pv_mms(kc):
                        for h in range(HC):
                            nc.tensor.matmul(
                                pvs[h],
                                lhsT=VA[
                                    :, kc,
                                    h * (VAW // 2) : h * (VAW // 2) + DP + 1,
                                ],
                                rhs=PT[:, kc % RING, h, :],
                                start=(kc == 0),
                                stop=(kc == KB - 1),
                            )

                    for kc in range(KB):
                        # PE order per window: PV(kc-2), fillers, S(kc) — the
                        # already-runnable work absorbs the wait for exp(kc-2)
                        # to release the S psum tile
                        if kc >= 2:
                            pv_mms(kc - 2)
                        # mandatory deadline pops, then one opportunistic pop
                        popped = False
                        while prep and prep[0][0] <= W:
                            prep.pop(0)[1]()
                            popped = True
                        if not popped and prep and kc >= 1:
                            prep.pop(0)[1]()
                        elif not popped and fillers and kc >= 2:
                            fillers.pop(0)()
                        # both heads' S matmuls share one psum tile and
                        # disjoint PE row-tiles -> hardware runs them
                        # concurrently; one exp evacuates both
                        ps2 = ps_s.tile([128, HC, 512], F32, tag="s", name="s2")
                        q0 = qh * QCH
                        for h in range(HC):
                            nc.tensor.matmul(
                                ps2[:, h, :],
                                lhsT=KT[
                                    h * DP : (h + 1) * DP,
                                    kc * 128 : (kc + 1) * 128,
                                ],
                                rhs=QT[h * DP : (h + 1) * DP, q0 : q0 + 512],
                                start=True,
                                stop=True,
                            )
                        nc.scalar.activation(
                            out=PT[:, kc % RING, :, :],
                            in_=ps2,
                            func=mybir.ActivationFunctionType.Exp,
                            scale=1.0 / SCALE,
                        )
                        W += 1
                    pv_mms(KB - 2)
                    pv_mms(KB - 1)

                    # normalize: denominator row is psum partition 0 (leading
                    # ones column). Evacuate pv on the scalar engine (slack at
                    # every boundary; frees the psum bank immediately), then
                    # fast-approx reciprocal, gpsimd broadcast, multiply
                    # (rows 0..64 for base-partition alignment; row 0 unused),
                    # DMA-shift into O^T.
                    for h in range(HC):
                        pv = pvs[h]
                        # reciprocal straight off the psum denominator row
                        # FIRST: the gpsimd broadcast then runs concurrently
                        # with the pv evacuation copy, shortening the chain
                        rt = normp.tile([1, QCH], F32, tag="rt", name="rt")
                        nc.vector.reciprocal_approx_fast(out=rt, in_=pv[0:1, :])
                        ocp = normp.tile([DP + 1, QCH], F32, tag="ocp", name="ocp")
                        nc.vector.tensor_copy(out=ocp, in_=pv)
                        bc = normp.tile([DP + 1, QCH], F32, tag="bc", name="bc")
                        nc.gpsimd.partition_broadcast(bc, rt)
                        ots = normp.tile([DP + 1, QCH], MM_DT, tag="ots", name="ots")
                        nc.vector.tensor_mul(out=ots, in0=ocp, in1=bc)
                        nc.sync.dma_start(
                            out=OT[h * DP : (h + 1) * DP, qh * QCH : (qh + 1) * QCH],
                            in_=ots[1 : DP + 1, :],
                        )

                    # queue this qh's projection blocks as fillers (their
                    # norm-chain inputs are ready well before they are popped)
                    for nb in range(qh * QCH // 128, (qh + 1) * QCH // 128):
                        fillers.append(
                            (lambda pool=None, s=(b, OT, nb): emit_proj_block(s, pool))
                        )

            # drain remaining fillers; pv accumulators are dead, so alternate
            # psum pools to keep 4 blocks in flight
            di = 0
            while fillers:
                fillers.pop(0)(ps_g if di % 2 else ps_y)
                di += 1
    nc.finalize()
    return nc


def _get_bass(with_bias=False):
    key = f"nc{int(with_bias)}"
    if key not in _CACHE:
        _CACHE[key] = _build_bass(with_bias)
    return _CACHE[key]


def _make_in_maps(x, W_qkv, b_qkv, W_out):
    """Shard the full inputs into the 8 per-core input dicts."""
    x = np.asarray(x, dtype=np.float32)
    W_qkv = np.asarray(W_qkv, dtype=np.float32)
    b_qkv = np.asarray(b_qkv, dtype=np.float32)
    W_out = np.asarray(W_out, dtype=np.float32)

    with_bias = bool(np.any(b_qkv))
    # x^T per batch, shared by all cores (+ optional bias chunk rows)
    xtt = x.transpose(0, 2, 1)
    if with_bias:
        aug = np.zeros((B, 128, N), dtype=np.float32)
        aug[:, 0, :] = 1.0
        xtt = np.concatenate([xtt, aug], axis=1)
    xt = np.ascontiguousarray(xtt).astype(BF16)

    in_maps = []
    for c in range(NCORES):
        heads = [HC * c + i for i in range(HC)]
        # W_qkv columns: head h occupies cols [h*3*DP, (h+1)*3*DP) as [q|k|v]
        qcols = [W_qkv[:, h * 3 * DP : h * 3 * DP + DP] for h in heads]
        kcols = [W_qkv[:, h * 3 * DP + DP : h * 3 * DP + 2 * DP] for h in heads]
        vcols = [W_qkv[:, h * 3 * DP + 2 * DP : h * 3 * DP + 3 * DP] for h in heads]
        wsel = np.concatenate(qcols + kcols + vcols, axis=1)  # [D, 3*E]
        if with_bias:
            bq = [b_qkv[h * 3 * DP : h * 3 * DP + DP] for h in heads]
            bk = [b_qkv[h * 3 * DP + DP : h * 3 * DP + 2 * DP] for h in heads]
            bv = [b_qkv[h * 3 * DP + 2 * DP : h * 3 * DP + 3 * DP] for h in heads]
            brow = np.concatenate(bq + bk + bv)  # [3*E]
            baug = np.zeros((128, 3 * E), dtype=np.float32)
            baug[0, :] = brow
            wsel = np.concatenate([wsel, baug], axis=0)
        woutc = np.concatenate(
            [W_out[h * DP : (h + 1) * DP, :] for h in heads], axis=0
        )  # [E, D]
        in_maps.append(
            {
                "xt": xt,
                "wsel": np.ascontiguousarray(wsel).astype(BF16),
                "wout": np.ascontiguousarray(woutc).astype(BF16),
            }
        )
    return in_maps, with_bias


def _run(in_maps, with_bias=False, trace=False):
    from concourse import bass_utils

    nc = _get_bass(with_bias)
    return bass_utils.run_bass_kernel_spmd(
        nc, in_maps, core_ids=list(range(NCORES)), trace=trace
    )


def kernel(x, W_qkv, b_qkv, W_out, b_out, _trace=False):
    in_maps, with_bias = _make_in_maps(x, W_qkv, b_qkv, W_out)
    res = _run(in_maps, with_bias=with_bias, trace=_trace)
    y = np.zeros((B, N, D), dtype=np.float32)
    for r in res.results:
        y += np.asarray(r["y"], dtype=np.float32)
    y += np.asarray(b_out, dtype=np.float32)
    _CACHE["last_result"] = res
    return y


# revision 40
# speedup vs baseline: 1.1320x; 1.0058x over previous
"""Multi-head self-attention on 8 Trainium2 NeuronCores.

Sharding: tensor-parallel over heads (2 heads per core, both batch elements
on every core). Each core computes qkv projection / attention / its slice of
the output projection (rows of W_out for its heads), producing a partial
[B, N, D] output (bf16); the host sums the 8 partials and adds b_out.

Per-core dataflow (layouts chosen so no engine ever needs a cross-partition
shift except via DMA):
  - host supplies x^T [B, D, N] so the QKV projection can run directly
    (contraction dim on partitions for both operands)
  - QKV^T = Wsel^T @ x^T -> Q^T, K^T, V^T, each [128=2*64 head rows, N]
  - V^T is PE-transposed back to V [k, e] chunks, with a LEADING ones column
    per head so the P@V matmul also produces the softmax row-sums, landing
    on psum partition 0 where the gpsimd broadcast can read them directly
  - S^T = K^T(head)^T-block @ Q^T (contraction = head dim 64). The two
    heads' S matmuls write one shared [128, 2, 512] psum tile and carry
    disjoint PE row-tiles (auto tile_position (0,0)/(64,0)), so the
    hardware runs them concurrently and one exp covers both heads.
  - P^T = exp(S^T / sqrt(dp)) fused in the PSUM->SBUF evacuation on ScalarE
    (no max subtraction: scores are ~N(0,1), exp is safe in fp32)
  - O^T_aug = [1|V]^T-block @ P^T -> row 0 = softmax denominator s, rows
    1..64 = unnormalized O^T; normalize via fast-approx reciprocal + gpsimd
    partition_broadcast + multiply, DMA-shift into the combined O^T tile
  - y_partial = O^T-block^T @ W_out_slice

QCH=512 keeps every psum user at 1-2 banks: 4 banks S double-buffer +
2 banks PV accumulators + 2 banks for filler work (projection blocks and
the deferred QKV pieces), so fillers never contend with the S ring.
Deferred prep is deadline-scheduled into the attention windows.
"""

import numpy as np
import ml_dtypes

B, N, D, H, DP = 2, 2048, 1024, 16, 64
SCALE = float(DP) ** 0.5
NCORES = 8
HC = H // NCORES            # heads per core = 2
E = HC * DP                 # per-core head-dim total = 128
QCH = 512                   # q columns handled per attention chunk
NQ = N // QCH               # 4
KB = N // 128               # 16 k blocks
DC = D // 128               # 8 contraction chunks for the qkv projection

BF16 = ml_dtypes.bfloat16

_CACHE = {}


def _build_bass(with_bias=False):
    import concourse.bass as bass
    import concourse.mybir as mybir
    import concourse.tile as tile
    from concourse import bacc
    from concourse.masks import make_identity

    MM_DT = mybir.dt.bfloat16    # matmul input dtype
    P_DT = mybir.dt.bfloat16     # exp(S^T) storage dtype
    F32 = mybir.dt.float32

    # nonzero b_qkv is handled by an extra contraction chunk whose x^T rows
    # are [ones, 0...] and whose weight rows carry the bias (bias as matmul)
    DCX = DC + (1 if with_bias else 0)
    VAW = 130  # VA free width: 2 heads x [ones | V(64)]
    RING = 4   # P^T ring depth (PV runs at lag 2)
    nc = bacc.Bacc(None, target_bir_lowering=False)
    xt = nc.dram_tensor("xt", [B, DCX * 128, N], MM_DT, kind="ExternalInput")[:]
    wsel = nc.dram_tensor("wsel", [DCX * 128, 3 * E], MM_DT, kind="ExternalInput")[:]
    wout = nc.dram_tensor("wout", [E, D], MM_DT, kind="ExternalInput")[:]
    # bf16 partials halve the output DMA; the host sums in fp32
    y = nc.dram_tensor("y", [B, N, D], MM_DT, kind="ExternalOutput")[:]

    with tile.TileContext(nc) as tc:
        with (
            tc.tile_pool(name="consts", bufs=1) as consts,
            tc.tile_pool(name="xtp", bufs=2) as xtp,
            tc.tile_pool(name="ptp", bufs=4) as ptp,
            tc.tile_pool(name="qkvp", bufs=2) as qkvp,
            tc.tile_pool(name="vap", bufs=2) as vap,
            tc.tile_pool(name="otp", bufs=2) as otp,
            tc.tile_pool(name="evacp", bufs=2) as evacp,
            tc.tile_pool(name="normp", bufs=2) as normp,
            # 8 psum banks total: paired-S 2x[128,2,512]f32 (4) +
            # pv accumulators 2x[65,512] (2) + filler scratch 2x[128,512] (2)
            tc.tile_pool(name="ps_s", bufs=2, space="PSUM") as ps_s,
            tc.tile_pool(name="ps_g", bufs=2, space="PSUM") as ps_g,
            tc.tile_pool(name="ps_y", bufs=2, space="PSUM") as ps_y,
        ):
            # DMA issue order matters: the Sync engine issues serially and the
            # first QKV matmul waits on xt[b=0] chunk 0 + WS, so those go
            # first; nk-halved transfers let the nk=0 projections (all the
            # attention start needs) complete in half the bytes
            XTs = []
            for b in range(B):
                XTs.append(xtp.tile([128, DCX, N], MM_DT, tag="xt", name="xt"))
            WS = consts.tile([128, DCX, 3 * E], MM_DT)
            wsr = wsel.rearrange("(dc p) e -> p dc e", p=128)
            xtb0 = xt[0].rearrange("(dc p) n -> p dc n", p=128)
            # the serial prep consumes only columns 0-511, so quarter-sized
            # first chunks let it start ~3us earlier
            for dc in range(DCX):
                nc.sync.dma_start(out=XTs[0][:, dc, 0:512], in_=xtb0[:, dc, 0:512])
                nc.sync.dma_start(out=WS[:, dc, :], in_=wsr[:, dc, :])
            for dc in range(DCX):
                nc.sync.dma_start(
                    out=XTs[0][:, dc, 512:1024], in_=xtb0[:, dc, 512:1024]
                )
            xtb1 = xt[1].rearrange("(dc p) n -> p dc n", p=128)
            for dc in range(DCX):
                nc.sync.dma_start(out=XTs[0][:, dc, 1024:], in_=xtb0[:, dc, 1024:])
            for nk in range(2):
                for dc in range(DCX):
                    nc.sync.dma_start(
                        out=XTs[1][:, dc, nk * 1024 : (nk + 1) * 1024],
                        in_=xtb1[:, dc, nk * 1024 : (nk + 1) * 1024],
                    )
            WOUT = consts.tile([128, D], MM_DT)
            nc.sync.dma_start(out=WOUT, in_=wout)
            IDENT = consts.tile([128, 128], MM_DT)
            make_identity(nc, IDENT)
            WARM = consts.tile([1, 1], F32)
            nc.vector.memset(WARM, 0.0)
            nc.scalar.activation(
                out=WARM, in_=WARM, func=mybir.ActivationFunctionType.Exp
            )
            # p-state warmup: the PE clock ramps 0.65 -> 1.2 -> 2.4 GHz after
            # ~3us of continuous busy. Junk transposes while the first x^T /
            # W chunks stream in mean the real matmuls start at full clock.
            WARMPS = ps_g.tile([128, 128], MM_DT, tag="g", name="warm_ps")
            for _ in range(24):
                nc.tensor.transpose(WARMPS, IDENT, IDENT)

            QKVTs, VAs = [], []
            fillers = []  # deferred projection sub-blocks (no deadline)
            for b in range(B):
                QKVTs.append(
                    [
                        qkvp.tile([128, N], MM_DT, tag=f"qkv{eb}", name=f"qkv{eb}")
                        for eb in range(3)
                    ]
                )
                # V chunks with a LEADING ones column: [1 | V_h0(64) | 1 | V_h1]
                VA = vap.tile([128, KB, VAW], MM_DT, tag="va", name="va")
                nc.gpsimd.memset(VA[:, :, 0:1], 1.0)
                nc.gpsimd.memset(VA[:, :, VAW // 2 : VAW // 2 + 1], 1.0)
                VAs.append(VA)

            def emit_qkv_half(b2, eb, nk, hf, pool):
                # half of a projection block: one 512-col chunk of QKV^T
                tag = "g" if pool is ps_g else "q"
                ps = pool.tile([128, 512], F32, tag=tag, name="ps_qkv")
                c0 = nk * 1024 + hf * 512
                for dc in range(DCX):
                    nc.tensor.matmul(
                        ps,
                        lhsT=WS[:, dc, eb * 128 : (eb + 1) * 128],
                        rhs=XTs[b2][:, dc, c0 : c0 + 512],
                        start=(dc == 0),
                        stop=(dc == DCX - 1),
                    )
                nc.vector.tensor_copy(
                    out=QKVTs[b2][eb][:, c0 : c0 + 512], in_=ps
                )

            def emit_vtrans(b2, kc, pool):
                tag = "g" if pool is ps_g else "q"
                pst = pool.tile([128, 128], MM_DT, tag=tag, name="ps_vt")
                VT2 = QKVTs[b2][2]
                VA2 = VAs[b2]
                nc.tensor.transpose(
                    pst, VT2[:, kc * 128 : (kc + 1) * 128], IDENT
                )
                nc.vector.tensor_copy(out=VA2[:, kc, 1 : 1 + DP], in_=pst[:, 0:DP])
                nc.vector.tensor_copy(
                    out=VA2[:, kc, VAW // 2 + 1 : VAW // 2 + 1 + DP],
                    in_=pst[:, DP : 2 * DP],
                )

            # ---- deferred-prep schedule. Window index W counts kc windows
            # globally (16 per qh, 64 per batch). Each prep item carries the
            # last window index at which it may be emitted (one before its
            # first reader); pops happen at the top of each window, so a
            # deadline of W is safe for readers inside window W+1.
            def deadlines(base, items):
                return [(base + dl, it) for dl, it in items]

            prep = []
            for b2 in range(B):
                base = 64 * b2
                items = []
                for nk in range(2):
                    for hf in range(2):
                        qi = 2 * nk + hf  # qh index using these Q cols
                        items.append(
                            (16 * qi - 2,
                             lambda b2=b2, nk=nk, hf=hf: emit_qkv_half(
                                 b2, 0, nk, hf, ps_y))
                        )
                        kf = 8 * nk + 4 * hf  # first k-block in these cols
                        items.append(
                            (kf - 2,
                             lambda b2=b2, nk=nk, hf=hf: emit_qkv_half(
                                 b2, 1, nk, hf, ps_y))
                        )
                        items.append(
                            (kf - 1,
                             lambda b2=b2, nk=nk, hf=hf: emit_qkv_half(
                                 b2, 2, nk, hf, ps_y))
                        )
                for kc in range(KB):
                    # first PV reader of VA[kc] is emitted in window kc+2
                    items.append(
                        (kc,
                         lambda b2=b2, kc=kc: emit_vtrans(b2, kc, ps_y))
                    )
                prep.extend(deadlines(base, items))
            prep.sort(key=lambda it: it[0])

            # b=0 items that would be due before the attention loop begins
            # run serially now (ps_g is free until the first pv allocation)
            while prep and prep[0][0] < 1:
                _, it = prep.pop(0)
                it()

            def emit_proj_block(spec, pool=None):
                b2, OT2, nb = spec
                pool = ps_y if pool is None else pool
                tag = "g" if pool is ps_g else "q"
                ysb = evacp.tile([128, D], MM_DT, tag="y", name="ysb", bufs=4)
                for dc2 in range(D // 512):
                    py = pool.tile([128, 512], F32, tag=tag, name="py")
                    nc.tensor.matmul(
                        py,
                        lhsT=OT2[:, nb * 128 : (nb + 1) * 128],
                        rhs=WOUT[:, dc2 * 512 : (dc2 + 1) * 512],
                        start=True,
                        stop=True,
                    )
                    nc.vector.tensor_copy(
                        out=ysb[:, dc2 * 512 : (dc2 + 1) * 512], in_=py
                    )
                nc.sync.dma_start(
                    out=y[b2, nb * 128 : (nb + 1) * 128, :], in_=ysb
                )

            # ---- phase 2: attention
            W = 0  # global window counter
            for b in range(B):
                QT, KT, VT = QKVTs[b]
                VA = VAs[b]
                OT = otp.tile([128, N], MM_DT, tag="ot", name="ot")
                for qh in range(NQ):
                    PT = ptp.tile(
                        [128, RING, HC, 512], P_DT, tag="pt", name="pt"
                    )
                    # one 1-bank PV accumulator per head, held across kc
                    pvs = [
                        ps_g.tile([DP + 1, 512], F32, tag="g", name=f"pv{h}")
                        for h in range(HC)
                    ]

                    def pv_mms(kc):
                        for h in range(HC):
                            nc.tensor.matmul(
                                pvs[h],
                                lhsT=VA[
                                    :, kc,
                                    h * (VAW // 2) : h * (VAW // 2) + DP + 1,
                                ],
                                rhs=PT[:, kc % RING, h, :],
                                start=(kc == 0),
                                stop=(kc == KB - 1),
                            )

                    for kc in range(KB):
                        # PE order per window: PV(kc-2), fillers, S(kc) — the
                        # already-runnable work absorbs the wait for exp(kc-2)
                        # to release the S psum tile
                        if kc >= 2:
                            pv_mms(kc - 2)
                        # mandatory deadline pops, then one opportunistic pop
                        popped = False
                        while prep and prep[0][0] <= W:
                            prep.pop(0)[1]()
                            popped = True
                        if not popped and prep and kc >= 1:
                            prep.pop(0)[1]()
                        elif not popped and fillers and kc >= 2:
                            fillers.pop(0)()
                        # both heads' S matmuls share one psum tile and
                        # disjoint PE row-tiles -> hardware runs them
                        # concurrently; one exp evacuates both
                        ps2 = ps_s.tile([128, HC, 512], F32, tag="s", name="s2")
                        q0 = qh * QCH
                        for h in range(HC):
                            nc.tensor.matmul(
                                ps2[:, h, :],
                                lhsT=KT[
                                    h * DP : (h + 1) * DP,
                                    kc * 128 : (kc + 1) * 128,
                                ],
                                rhs=QT[h * DP : (h + 1) * DP, q0 : q0 + 512],
                                start=True,
                                stop=True,
                            )
                        nc.scalar.activation(
                            out=PT[:, kc % RING, :, :],
                            in_=ps2,
                            func=mybir.ActivationFunctionType.Exp,
                            scale=1.0 / SCALE,
                        )
                        W += 1
                    pv_mms(KB - 2)
                    pv_mms(KB - 1)

                    # normalize: denominator row is psum partition 0 (leading
                    # ones column). Evacuate pv on the scalar engine (slack at
                    # every boundary; frees the psum bank immediately), then
                    # fast-approx reciprocal, gpsimd broadcast, multiply
                    # (rows 0..64 for base-partition alignment; row 0 unused),
                    # DMA-shift into O^T.
                    for h in range(HC):
                        pv = pvs[h]
                        # reciprocal straight off the psum denominator row
                        # FIRST: the gpsimd broadcast then runs concurrently
                        # with the pv evacuation copy, shortening the chain
                        rt = normp.tile([1, QCH], F32, tag="rt", name="rt")
                        nc.vector.reciprocal_approx_fast(out=rt, in_=pv[0:1, :])
                        ocp = normp.tile([DP + 1, QCH], F32, tag="ocp", name="ocp")
                        nc.vector.tensor_copy(out=ocp, in_=pv)
                        bc = normp.tile([DP + 1, QCH], F32, tag="bc", name="bc")
                        nc.gpsimd.partition_broadcast(bc, rt)
                        ots = normp.tile([DP + 1, QCH], MM_DT, tag="ots", name="ots")
                        nc.vector.tensor_mul(out=ots, in0=ocp, in1=bc)
                        nc.sync.dma_start(
                            out=OT[h * DP : (h + 1) * DP, qh * QCH : (qh + 1) * QCH],
                            in_=ots[1 : DP + 1, :],
                        )

                    # queue this qh's projection blocks as fillers (their
                    # norm-chain inputs are ready well before they are popped)
                    for nb in range(qh * QCH // 128, (qh + 1) * QCH // 128):
                        fillers.append(
                            (lambda pool=None, s=(b, OT, nb): emit_proj_block(s, pool))
                        )

            # drain remaining fillers; pv accumulators are dead, so alternate
            # psum pools to keep 4 blocks in flight
            di = 0
            while fillers:
                fillers.pop(0)(ps_g if di % 2 else ps_y)
                di += 1
    nc.finalize()
    return nc


def _get_bass(with_bias=False):
    key = f"nc{int(with_bias)}"
    if key not in _CACHE:
        _CACHE[key] = _build_bass(with_bias)
    return _CACHE[key]


def _make_in_maps(x, W_qkv, b_qkv, W_out):
    """Shard the full inputs into the 8 per-core input dicts."""
    x = np.asarray(x, dtype=np.float32)
    W_qkv = np.asarray(W_qkv, dtype=np.float32)
    b_qkv = np.asarray(b_qkv, dtype=np.float32)
    W_out = np.asarray(W_out, dtype=np.float32)

    with_bias = bool(np.any(b_qkv))
    # x^T per batch, shared by all cores (+ optional bias chunk rows)
    xtt = x.transpose(0, 2, 1)
    if with_bias:
        aug = np.zeros((B, 128, N), dtype=np.float32)
        aug[:, 0, :] = 1.0
        xtt = np.concatenate([xtt, aug], axis=1)
    xt = np.ascontiguousarray(xtt).astype(BF16)

    in_maps = []
    for c in range(NCORES):
        heads = [HC * c + i for i in range(HC)]
        # W_qkv columns: head h occupies cols [h*3*DP, (h+1)*3*DP) as [q|k|v]
        qcols = [W_qkv[:, h * 3 * DP : h * 3 * DP + DP] for h in heads]
        kcols = [W_qkv[:, h * 3 * DP + DP : h * 3 * DP + 2 * DP] for h in heads]
        vcols = [W_qkv[:, h * 3 * DP + 2 * DP : h * 3 * DP + 3 * DP] for h in heads]
        wsel = np.concatenate(qcols + kcols + vcols, axis=1)  # [D, 3*E]
        if with_bias:
            bq = [b_qkv[h * 3 * DP : h * 3 * DP + DP] for h in heads]
            bk = [b_qkv[h * 3 * DP + DP : h * 3 * DP + 2 * DP] for h in heads]
            bv = [b_qkv[h * 3 * DP + 2 * DP : h * 3 * DP + 3 * DP] for h in heads]
            brow = np.concatenate(bq + bk + bv)  # [3*E]
            baug = np.zeros((128, 3 * E), dtype=np.float32)
            baug[0, :] = brow
            wsel = np.concatenate([wsel, baug], axis=0)
        woutc = np.concatenate(
            [W_out[h * DP : (h + 1) * DP, :] for h in heads], axis=0
        )  # [E, D]
        in_maps.append(
            {
                "xt": xt,
                "wsel": np.ascontiguousarray(wsel).astype(BF16),
                "wout": np.ascontiguousarray(woutc).astype(BF16),
            }
        )
    return in_maps, with_bias


def _run(in_maps, with_bias=False, trace=False):
    from concourse import bass_utils

    nc = _get_bass(with_bias)
    return bass_utils.run_bass_kernel_spmd(
        nc, in_maps, core_ids=list(range(NCORES)), trace=trace
    )


def kernel(x, W_qkv, b_qkv, W_out, b_out, _trace=False):
    in_maps, with_bias = _make_in_maps(x, W_qkv, b_qkv, W_out)
    res = _run(in_maps, with_bias=with_bias, trace=_trace)
    y = np.zeros((B, N, D), dtype=np.float32)
    for r in res.results:
        y += np.asarray(r["y"], dtype=np.float32)
    y += np.asarray(b_out, dtype=np.float32)
    _CACHE["last_result"] = res
    return y
